# Initial kernel scaffold
#
"""Grouped MoE MLP (SwiGLU) kernel for Trainium2, 8 NeuronCores.

Strategy (expert-parallel, host-side routing):
  Tokens arrive pre-sorted by expert with per-expert counts.  The host
  partitions each expert's token block into pieces matching a fixed
  per-core slot structure (preferring the zero-padding mixed cover
  (1024, 512, 256, 256) = 2048 rows/core, falling back to a uniform
  768-row scheme), and gathers the matching expert weights per
  (core, slot).  Every core runs the identical program: for each slot,
  a dense SwiGLU MLP of that slot's tokens with that slot's expert
  weights.  No device-side routing or collectives are needed.

  Layouts are transposed on the host so both GEMMs contract over the
  SBUF partition dimension with no on-chip transposes:
    GEMM1: out1^T[f, t] = sum_h W1[h, f] * x[t, h]   (h on partitions)
    SwiGLU on feature-partitioned tiles
    GEMM2: out^T[o, t]  = sum_f W2[f, o] * h[t, f]   (f on partitions)
"""

import math
from contextlib import ExitStack

import ml_dtypes
import numpy as np

P = 128
HIDDEN = 2048
INTER = 1408
GU = 2 * INTER            # 2816 = gate+up columns
KH = HIDDEN // P          # 16 k-tiles for GEMM1
KI = INTER // P           # 11 k-tiles for GEMM2 / gate-up pair blocks
MO = HIDDEN // P          # 16 output feature blocks
N_CORES = 8
NT = 512                  # max tokens per chunk (matmul moving free dim)
MIXED_SLOTS = (1024, 512, 256, 256)   # zero-padding cover, 2048 rows/core
UNIFORM_SLOT = 768                    # fallback slot size

BF16 = ml_dtypes.bfloat16

_PROGRAM_CACHE: dict = {}


def _chunks(slot_rows: int, nt: int):
    out = []
    r = 0
    while r < slot_rows:
        c = min(nt, slot_rows - r)
        out.append((r, c))
        r += c
    return out


def _build_program(slot_sizes: tuple, nt: int):
    import concourse.mybir as mybir
    import concourse.tile as tile
    from concourse import bacc

    n_slots = len(slot_sizes)
    T = sum(slot_sizes)
    slot_off = np.concatenate([[0], np.cumsum(slot_sizes)]).astype(int)
    bf16 = mybir.dt.bfloat16
    f32 = mybir.dt.float32

    nc = bacc.Bacc(None, target_bir_lowering=False, debug=False)
    xT = nc.dram_tensor("xT", [P, KH, T], bf16, kind="ExternalInput")
    w1 = nc.dram_tensor("w1", [n_slots, P, KH, GU], bf16, kind="ExternalInput")
    w2 = nc.dram_tensor("w2", [n_slots, P, KI, HIDDEN], bf16, kind="ExternalInput")
    outT = nc.dram_tensor("outT", [P, MO, T], f32, kind="ExternalOutput")

    with tile.TileContext(nc) as tc, ExitStack() as ctx:
        w1_pool = ctx.enter_context(tc.tile_pool(name="w1p", bufs=1))
        w2_pool = ctx.enter_context(tc.tile_pool(name="w2p", bufs=1))
        x_pool = ctx.enter_context(tc.tile_pool(name="xp", bufs=2))
        h_pool = ctx.enter_context(tc.tile_pool(name="hp", bufs=2))
        g_pool = ctx.enter_context(tc.tile_pool(name="gp", bufs=2))
        o_pool = ctx.enter_context(tc.tile_pool(name="op", bufs=4))
        ps1 = ctx.enter_context(tc.tile_pool(name="ps1", bufs=2, space="PSUM"))
        ps2 = ctx.enter_context(tc.tile_pool(name="ps2", bufs=2, space="PSUM"))

        for s in range(n_slots):
            chunk_list = _chunks(slot_sizes[s], nt)
            # first x chunk of the slot goes ahead of the weight DMAs so the
            # first GEMM1 matmul only waits for w1[k=0], not the whole slot's
            # weights (HWDGE queues are FIFO)
            c0_off, c0_n = chunk_list[0]
            xt0 = x_pool.tile([P, KH, c0_n], bf16, tag="xt")
            t00 = int(slot_off[s]) + c0_off
            nc.sync.dma_start(xt0[:], xT[:, :, t00 : t00 + c0_n])
            w1t = w1_pool.tile([P, KH, GU], bf16)
            for k in range(KH):
                nc.sync.dma_start(w1t[:, k, :], w1[s, :, k, :])
            w2t = w2_pool.tile([P, KI, HIDDEN], bf16)
            for k in range(KI):
                nc.sync.dma_start(w2t[:, k, :], w2[s, :, k, :])
            for ci, (c_off, c_n) in enumerate(chunk_list):
                t0 = int(slot_off[s]) + c_off
                if ci == 0:
                    xt = xt0
                else:
                    xt = x_pool.tile([P, KH, c_n], bf16, tag="xt")
                    nc.sync.dma_start(xt[:], xT[:, :, t0 : t0 + c_n])
                ht = h_pool.tile([P, KI, c_n], bf16, tag="ht")
                for mp in range(KI):
                    pg = ps1.tile([P, c_n], f32, tag="pg")
                    pu = ps1.tile([P, c_n], f32, tag="pu")
                    for k in range(KH):
                        nc.tensor.matmul(
                            pg[:],
                            w1t[:, k, mp * P : (mp + 1) * P],
                            xt[:, k, :],
                            start=(k == 0),
                            stop=(k == KH - 1),
                        )
                    for k in range(KH):
                        nc.tensor.matmul(
                            pu[:],
                            w1t[:, k, (KI + mp) * P : (KI + mp + 1) * P],
                            xt[:, k, :],
                            start=(k == 0),
                            stop=(k == KH - 1),
                        )
                    gt = g_pool.tile([P, c_n], bf16, tag="gt")
                    nc.scalar.activation(
                        gt[:], pg[:], mybir.ActivationFunctionType.Silu
                    )
                    nc.vector.tensor_mul(ht[:, mp, :], gt[:], pu[:])
                for m in range(MO):
                    po = ps2.tile([P, c_n], f32, tag="po")
                    for k in range(KI):
                        nc.tensor.matmul(
                            po[:],
                            w2t[:, k, m * P : (m + 1) * P],
                            ht[:, k, :],
                            start=(k == 0),
                            stop=(k == KI - 1),
                        )
                    om = o_pool.tile([P, c_n], f32, tag="om")
                    nc.vector.tensor_copy(om[:], po[:])
                    nc.sync.dma_start(outT[:, m, t0 : t0 + c_n], om[:])
    nc.compile()
    return nc


def _get_program(slot_sizes: tuple, nt: int):
    key = (tuple(slot_sizes), nt)
    if key not in _PROGRAM_CACHE:
        _PROGRAM_CACHE[key] = _build_program(tuple(slot_sizes), nt)
    return _PROGRAM_CACHE[key]


def _pack_w1(w: np.ndarray) -> np.ndarray:
    # [HIDDEN, GU] f32 -> [P, KH, GU] bf16 with row h = 128*k + p
    return np.ascontiguousarray(
        w.reshape(KH, P, GU).transpose(1, 0, 2).astype(BF16)
    )


def _pack_w2(w: np.ndarray) -> np.ndarray:
    # [INTER, HIDDEN] f32 -> [P, KI, HIDDEN] bf16 with row f = 128*k + p
    return np.ascontiguousarray(
        w.reshape(KI, P, HIDDEN).transpose(1, 0, 2).astype(BF16)
    )


def _mixed_cover(counts, slot_sizes):
    """Exact-cover counts by pieces {size: N_CORES per size}. Returns
    per-core shard lists [(expert, row0, nrows), ...] ordered like
    slot_sizes, or None if no exact cover exists."""
    from collections import Counter

    sizes_desc = sorted(slot_sizes, reverse=True)
    avail = Counter(slot_sizes)
    for s in avail:
        avail[s] *= N_CORES
    per_expert: list = [None] * len(counts)

    def cover(rem, max_size):
        if rem == 0:
            return []
        for s in sorted(set(avail), reverse=True):
            if s > max_size or s > rem or avail[s] == 0:
                continue
            avail[s] -= 1
            sub = cover(rem - s, s)
            if sub is not None:
                return [s] + sub
            avail[s] += 1
        return None

    # Largest counts first so big pieces go where they must.
    order = sorted(range(len(counts)), key=lambda e: -counts[e])
    for e in order:
        pieces = cover(counts[e], max(sizes_desc))
        if pieces is None:
            return None
        per_expert[e] = pieces

    # Build shard pieces and deal them out per size class.
    by_size: dict = {s: [] for s in set(slot_sizes)}
    for e in range(len(counts)):
        r = 0
        for s in sorted(per_expert[e], reverse=True):
            by_size[s].append((e, r, s))
            r += s
    # Pad classes with empty shards (possible when sum(counts) is short).
    for s, lst in by_size.items():
        want = slot_sizes.count(s) * N_CORES
        while len(lst) < want:
            lst.append((0, 0, 0))
        if len(lst) != want:
            return None

    cores = []
    for r in range(N_CORES):
        shards = []
        used = {s: 0 for s in by_size}
        for s in slot_sizes:
            shards.append(by_size[s][r * slot_sizes.count(s) + used[s]])
            used[s] += 1
        cores.append(shards)
    return cores


def _uniform_cover(counts, slot):
    shards = []
    for e in range(len(counts)):
        r = 0
        while r < counts[e]:
            n = min(slot, counts[e] - r)
            shards.append((e, r, n))
            r += n
    n_slots = max(1, math.ceil(len(shards) / N_CORES))
    while len(shards) < N_CORES * n_slots:
        shards.append((0, 0, 0))
    return [shards[r * n_slots : (r + 1) * n_slots] for r in range(N_CORES)], n_slots


def _run(
    hidden_states: np.ndarray,
    merged_gate_up_proj: np.ndarray,
    merged_down_proj: np.ndarray,
    num_tokens_per_expert: np.ndarray,
    trace: bool = False,
):
    from concourse.bass_utils import run_bass_kernel_spmd

    counts = [int(c) for c in np.asarray(num_tokens_per_expert)]
    n_experts = len(counts)
    offs = np.concatenate([[0], np.cumsum(counts)]).astype(int)
    total = int(offs[-1])

    core_shards = _mixed_cover(counts, MIXED_SLOTS)
    if core_shards is not None:
        slot_sizes = MIXED_SLOTS
    else:
        core_shards, n_slots = _uniform_cover(counts, UNIFORM_SLOT)
        slot_sizes = (UNIFORM_SLOT,) * n_slots

    slot_off = np.concatenate([[0], np.cumsum(slot_sizes)]).astype(int)
    T = int(slot_off[-1])

    nc = _get_program(slot_sizes, NT)

    w1_packed = [_pack_w1(merged_gate_up_proj[e]) for e in range(n_experts)]
    w2_packed = [_pack_w2(merged_down_proj[e]) for e in range(n_experts)]
    x_bf16 = hidden_states.astype(BF16)

    in_maps = []
    for r in range(N_CORES):
        shards = core_shards[r]
        x_core = np.zeros((T, HIDDEN), dtype=BF16)
        for s, (e, r0, n) in enumerate(shards):
            if n:
                x_core[slot_off[s] : slot_off[s] + n] = x_bf16[
                    offs[e] + r0 : offs[e] + r0 + n
                ]
        # [T, HIDDEN] -> [P, KH, T] with column h = 128*k + p
        xT_core = np.ascontiguousarray(
            x_core.T.reshape(KH, P, T).transpose(1, 0, 2)
        )
        in_maps.append(
            {
                "xT": xT_core,
                "w1": np.stack([w1_packed[e] for (e, _, _) in shards]),
                "w2": np.stack([w2_packed[e] for (e, _, _) in shards]),
            }
        )

    res = run_bass_kernel_spmd(nc, in_maps, list(range(N_CORES)), trace=trace)

    out = np.empty((total, HIDDEN), dtype=np.float32)
    for r in range(N_CORES):
        # [P, MO, T] -> [T, HIDDEN] with column o = 128*m + p
        o_core = res.results[r]["outT"].transpose(2, 1, 0).reshape(T, HIDDEN)
        for s, (e, r0, n) in enumerate(core_shards[r]):
            if n:
                out[offs[e] + r0 : offs[e] + r0 + n] = o_core[
                    slot_off[s] : slot_off[s] + n
                ]
    return out, res


def kernel(**inputs) -> np.ndarray:
    return _run(**inputs, trace=False)[0]


def run_traced(**inputs):
    return _run(**inputs, trace=True)



# revision 1
# speedup vs baseline: 1.0590x; 1.0590x over previous
"""Grouped MoE MLP (SwiGLU) kernel for Trainium2, 8 NeuronCores.

Strategy (expert-parallel, host-side routing):
  Tokens arrive pre-sorted by expert with per-expert counts.  The host
  partitions each expert's token block into pieces matching a fixed
  per-core slot structure (preferring the zero-padding mixed cover
  (1024, 512, 256, 256) = 2048 rows/core, falling back to a uniform
  768-row scheme), and gathers the matching expert weights per
  (core, slot).  Every core runs the identical program: for each slot,
  a dense SwiGLU MLP of that slot's tokens with that slot's expert
  weights.  No device-side routing or collectives are needed.

  Layouts are transposed on the host so both GEMMs contract over the
  SBUF partition dimension with no on-chip transposes:
    GEMM1: out1^T[f, t] = sum_h W1[h, f] * x[t, h]   (h on partitions)
    SwiGLU on feature-partitioned tiles
    GEMM2: out^T[o, t]  = sum_f W2[f, o] * h[t, f]   (f on partitions)
"""

import math
from contextlib import ExitStack

import ml_dtypes
import numpy as np

P = 128
HIDDEN = 2048
INTER = 1408
GU = 2 * INTER            # 2816 = gate+up columns
KH = HIDDEN // P          # 16 k-tiles for GEMM1
KI = INTER // P           # 11 k-tiles for GEMM2 / gate-up pair blocks
MO = HIDDEN // P          # 16 output feature blocks
N_CORES = 8
NT = 512                  # max tokens per chunk (matmul moving free dim)
MIXED_SLOTS = (1024, 512, 256, 256)   # zero-padding cover, 2048 rows/core
UNIFORM_SLOT = 768                    # fallback slot size

BF16 = ml_dtypes.bfloat16

_PROGRAM_CACHE: dict = {}


def _chunks(slot_rows: int, nt: int):
    out = []
    r = 0
    while r < slot_rows:
        c = min(nt, slot_rows - r)
        out.append((r, c))
        r += c
    return out


def _build_program(slot_sizes: tuple, nt: int):
    import concourse.mybir as mybir
    import concourse.tile as tile
    from concourse import bacc

    n_slots = len(slot_sizes)
    T = sum(slot_sizes)
    slot_off = np.concatenate([[0], np.cumsum(slot_sizes)]).astype(int)
    bf16 = mybir.dt.bfloat16
    f32 = mybir.dt.float32

    nc = bacc.Bacc(None, target_bir_lowering=False, debug=False)
    xT = nc.dram_tensor("xT", [P, KH, T], bf16, kind="ExternalInput")
    w1 = nc.dram_tensor("w1", [n_slots, P, KH, GU], bf16, kind="ExternalInput")
    w2 = nc.dram_tensor("w2", [n_slots, P, KI, HIDDEN], bf16, kind="ExternalInput")
    outT = nc.dram_tensor("outT", [P, MO, T], f32, kind="ExternalOutput")

    with tile.TileContext(nc) as tc, ExitStack() as ctx:
        w1_pool = ctx.enter_context(tc.tile_pool(name="w1p", bufs=1))
        w2_pool = ctx.enter_context(tc.tile_pool(name="w2p", bufs=1))
        x_pool = ctx.enter_context(tc.tile_pool(name="xp", bufs=2))
        h_pool = ctx.enter_context(tc.tile_pool(name="hp", bufs=2))
        g_pool = ctx.enter_context(tc.tile_pool(name="gp", bufs=2))
        o_pool = ctx.enter_context(tc.tile_pool(name="op", bufs=4))
        ps1 = ctx.enter_context(tc.tile_pool(name="ps1", bufs=2, space="PSUM"))
        ps2 = ctx.enter_context(tc.tile_pool(name="ps2", bufs=2, space="PSUM"))

        for s in range(n_slots):
            chunk_list = _chunks(slot_sizes[s], nt)
            # first x chunk of the slot goes ahead of the weight DMAs so the
            # first GEMM1 matmul only waits for w1[k=0], not the whole slot's
            # weights (HWDGE queues are FIFO)
            c0_off, c0_n = chunk_list[0]
            xt0 = x_pool.tile([P, KH, c0_n], bf16, tag="xt")
            t00 = int(slot_off[s]) + c0_off
            nc.sync.dma_start(xt0[:], xT[:, :, t00 : t00 + c0_n])
            w1t = w1_pool.tile([P, KH, GU], bf16)
            for k in range(KH):
                nc.sync.dma_start(w1t[:, k, :], w1[s, :, k, :])
            w2t = w2_pool.tile([P, KI, HIDDEN], bf16)
            for k in range(KI):
                nc.sync.dma_start(w2t[:, k, :], w2[s, :, k, :])
            for ci, (c_off, c_n) in enumerate(chunk_list):
                t0 = int(slot_off[s]) + c_off
                if ci == 0:
                    xt = xt0
                else:
                    xt = x_pool.tile([P, KH, c_n], bf16, tag="xt")
                    nc.sync.dma_start(xt[:], xT[:, :, t0 : t0 + c_n])
                ht = h_pool.tile([P, KI, c_n], bf16, tag="ht")
                for mp in range(KI):
                    pg = ps1.tile([P, c_n], f32, tag="pg")
                    pu = ps1.tile([P, c_n], f32, tag="pu")
                    for k in range(KH):
                        nc.tensor.matmul(
                            pg[:],
                            w1t[:, k, mp * P : (mp + 1) * P],
                            xt[:, k, :],
                            start=(k == 0),
                            stop=(k == KH - 1),
                        )
                    for k in range(KH):
                        nc.tensor.matmul(
                            pu[:],
                            w1t[:, k, (KI + mp) * P : (KI + mp + 1) * P],
                            xt[:, k, :],
                            start=(k == 0),
                            stop=(k == KH - 1),
                        )
                    gt = g_pool.tile([P, c_n], bf16, tag="gt")
                    nc.scalar.activation(
                        gt[:], pg[:], mybir.ActivationFunctionType.Silu
                    )
                    nc.vector.tensor_mul(ht[:, mp, :], gt[:], pu[:])
                for m in range(MO):
                    po = ps2.tile([P, c_n], f32, tag="po")
                    for k in range(KI):
                        nc.tensor.matmul(
                            po[:],
                            w2t[:, k, m * P : (m + 1) * P],
                            ht[:, k, :],
                            start=(k == 0),
                            stop=(k == KI - 1),
                        )
                    om = o_pool.tile([P, c_n], f32, tag="om")
                    nc.vector.tensor_copy(om[:], po[:])
                    nc.sync.dma_start(outT[:, m, t0 : t0 + c_n], om[:])
    nc.compile()
    return nc


def _get_program(slot_sizes: tuple, nt: int):
    key = (tuple(slot_sizes), nt)
    if key not in _PROGRAM_CACHE:
        _PROGRAM_CACHE[key] = _build_program(tuple(slot_sizes), nt)
    return _PROGRAM_CACHE[key]


def _pack_w1(w: np.ndarray) -> np.ndarray:
    # [HIDDEN, GU] f32 -> [P, KH, GU] bf16 with row h = 128*k + p
    return np.ascontiguousarray(
        w.reshape(KH, P, GU).transpose(1, 0, 2).astype(BF16)
    )


def _pack_w2(w: np.ndarray) -> np.ndarray:
    # [INTER, HIDDEN] f32 -> [P, KI, HIDDEN] bf16 with row f = 128*k + p
    return np.ascontiguousarray(
        w.reshape(KI, P, HIDDEN).transpose(1, 0, 2).astype(BF16)
    )


def _mixed_cover(counts, slot_sizes):
    """Exact-cover counts by pieces {size: N_CORES per size}. Returns
    per-core shard lists [(expert, row0, nrows), ...] ordered like
    slot_sizes, or None if no exact cover exists."""
    from collections import Counter

    sizes_desc = sorted(slot_sizes, reverse=True)
    avail = Counter(slot_sizes)
    for s in avail:
        avail[s] *= N_CORES
    per_expert: list = [None] * len(counts)

    def cover(rem, max_size):
        if rem == 0:
            return []
        for s in sorted(set(avail), reverse=True):
            if s > max_size or s > rem or avail[s] == 0:
                continue
            avail[s] -= 1
            sub = cover(rem - s, s)
            if sub is not None:
                return [s] + sub
            avail[s] += 1
        return None

    # Largest counts first so big pieces go where they must.
    order = sorted(range(len(counts)), key=lambda e: -counts[e])
    for e in order:
        pieces = cover(counts[e], max(sizes_desc))
        if pieces is None:
            return None
        per_expert[e] = pieces

    # Build shard pieces and deal them out per size class.
    by_size: dict = {s: [] for s in set(slot_sizes)}
    for e in range(len(counts)):
        r = 0
        for s in sorted(per_expert[e], reverse=True):
            by_size[s].append((e, r, s))
            r += s
    # Pad classes with empty shards (possible when sum(counts) is short).
    for s, lst in by_size.items():
        want = slot_sizes.count(s) * N_CORES
        while len(lst) < want:
            lst.append((0, 0, 0))
        if len(lst) != want:
            return None

    cores = []
    for r in range(N_CORES):
        shards = []
        used = {s: 0 for s in by_size}
        for s in slot_sizes:
            shards.append(by_size[s][r * slot_sizes.count(s) + used[s]])
            used[s] += 1
        cores.append(shards)
    return cores


def _uniform_cover(counts, slot):
    shards = []
    for e in range(len(counts)):
        r = 0
        while r < counts[e]:
            n = min(slot, counts[e] - r)
            shards.append((e, r, n))
            r += n
    n_slots = max(1, math.ceil(len(shards) / N_CORES))
    while len(shards) < N_CORES * n_slots:
        shards.append((0, 0, 0))
    return [shards[r * n_slots : (r + 1) * n_slots] for r in range(N_CORES)], n_slots


def _run(
    hidden_states: np.ndarray,
    merged_gate_up_proj: np.ndarray,
    merged_down_proj: np.ndarray,
    num_tokens_per_expert: np.ndarray,
    trace: bool = False,
):
    from concourse.bass_utils import run_bass_kernel_spmd

    counts = [int(c) for c in np.asarray(num_tokens_per_expert)]
    n_experts = len(counts)
    offs = np.concatenate([[0], np.cumsum(counts)]).astype(int)
    total = int(offs[-1])

    core_shards = _mixed_cover(counts, MIXED_SLOTS)
    if core_shards is not None:
        slot_sizes = MIXED_SLOTS
    else:
        core_shards, n_slots = _uniform_cover(counts, UNIFORM_SLOT)
        slot_sizes = (UNIFORM_SLOT,) * n_slots

    slot_off = np.concatenate([[0], np.cumsum(slot_sizes)]).astype(int)
    T = int(slot_off[-1])

    nc = _get_program(slot_sizes, NT)

    w1_packed = [_pack_w1(merged_gate_up_proj[e]) for e in range(n_experts)]
    w2_packed = [_pack_w2(merged_down_proj[e]) for e in range(n_experts)]
    x_bf16 = hidden_states.astype(BF16)

    in_maps = []
    for r in range(N_CORES):
        shards = core_shards[r]
        x_core = np.zeros((T, HIDDEN), dtype=BF16)
        for s, (e, r0, n) in enumerate(shards):
            if n:
                x_core[slot_off[s] : slot_off[s] + n] = x_bf16[
                    offs[e] + r0 : offs[e] + r0 + n
                ]
        # [T, HIDDEN] -> [P, KH, T] with column h = 128*k + p
        xT_core = np.ascontiguousarray(
            x_core.T.reshape(KH, P, T).transpose(1, 0, 2)
        )
        in_maps.append(
            {
                "xT": xT_core,
                "w1": np.stack([w1_packed[e] for (e, _, _) in shards]),
                "w2": np.stack([w2_packed[e] for (e, _, _) in shards]),
            }
        )

    res = run_bass_kernel_spmd(nc, in_maps, list(range(N_CORES)), trace=trace)

    out = np.empty((total, HIDDEN), dtype=np.float32)
    for r in range(N_CORES):
        # [P, MO, T] -> [T, HIDDEN] with column o = 128*m + p
        o_core = res.results[r]["outT"].transpose(2, 1, 0).reshape(T, HIDDEN)
        for s, (e, r0, n) in enumerate(core_shards[r]):
            if n:
                out[offs[e] + r0 : offs[e] + r0 + n] = o_core[
                    slot_off[s] : slot_off[s] + n
                ]
    return out, res


def kernel(**inputs) -> np.ndarray:
    return _run(**inputs, trace=False)[0]


def run_traced(**inputs):
    return _run(**inputs, trace=True)

